# revision 31
# baseline (speedup 1.0000x reference)
"""Causal self-attention with RoPE, fused Trainium2 Bass kernel, 8 NeuronCores.

Problem: x[4,2048,1024] @ W_attn[1024,3072] -> qkv; RoPE(q,k); causal
softmax attention (16 heads, d=64); y @ W_proj[1024,1024].

Sharding (data + head parallel): core c handles batch b=c//2 and heads
8*(c%2)..8*(c%2)+7.  W_attn is column-sharded by head, W_proj row-sharded;
each core emits a partial output projection and the host sums the two
partials per batch (the 2-way "all-reduce").

Kernel layout choices (per core):
 - Everything transposed: xT [D,T] in SBUF, q/k produced as qT/kT [d,T],
   attention computed as scoresT [k,q] so softmax-sum and PV contraction
   both run along the partition axis via matmuls (no transposes needed).
 - RoPE: head-dim channels are pre-permuted (via W_attn column permutation)
   into [e0..e15, o0..o15, e16..e31, o16..o31] per head so the rotate-half
   pairing is a 16<->16 swap inside each 32-partition quadrant, done with a
   single DVE stream_shuffle.  cos/sin tables (sign-folded) come from host.
 - No max-subtraction in softmax: scores/8 are ~N(0,0.4), exp is safe.
   l (row sum) comes free by appending a ones column to V (M=65 PV matmul).
 - fp16 operands everywhere on the PE (full-rate); fp32 accumulation.
 - Scores matmuls for a head pair run concurrently via row-tiled PE
   (stationaries at base partitions 0/64, separate PSUM banks).

Scheduling (v2, PE-bound regime):
 - All of phase A-v (x@Wv) runs upfront: the ACT engine is idle then
   anyway, and it frees its PSUM bank for the window phase.
 - The ACT queue carries ONLY the softmax exps; everything else
   (copies, casts) lives on DVE/GPSIMD so window boundaries never stall
   the exp stream.
 - Attention windows are software-pipelined: score(kb+1) is emitted
   before PV(kb), with filler matmuls (remaining q/k projection units,
   out-projection units) dripped between score and PV so the PE chews
   dense work while ACT runs exp.
 - Normalization: 1/l via DVE reciprocal straight from PSUM row 64,
   broadcast across partitions with a GPSIMD partition_broadcast, then a
   single fused DVE multiply (PSUM y x SBUF r -> SBUF f16).  No PE
   broadcast matmul, nothing on ACT.
 - Output partials are f16 (halves the store DMA); host sums in f32.
"""

import sys

sys.path.insert(0, "/opt/trn_rl_repo")

import numpy as np

import concourse.bass as bass  # noqa: F401  (import registers engine classes)
import concourse.mybir as mybir
import concourse.tile as tile
from concourse import bacc
from concourse.bass_utils import run_bass_kernel_spmd

F16 = mybir.dt.float16
F32 = mybir.dt.float32

B, T, D = 4, 2048, 1024
N_HEAD, D_HEAD = 16, 64
ROPE_BASE = 10000.0
N_CORES = 8
HPC = N_HEAD // 2  # heads per core (8)
NPAIR = HPC // 2  # head pairs per core (4)
NKC = D // 128  # k-chunks (8)
NQC = T // 512  # q chunks of 512 (4)
NKB = T // 128  # k blocks of 128 (16)

SWAP_MASK = list(range(16, 32)) + list(range(0, 16))


def _build_program():
    nc = bacc.Bacc("TRN2", target_bir_lowering=False, debug=False,
                   num_devices=N_CORES)

    xT_d = nc.dram_tensor("xT", [D, T], F16, kind="ExternalInput").ap()
    wqk_d = nc.dram_tensor("wqk", [D, 1024], F16, kind="ExternalInput").ap()
    wv_d = nc.dram_tensor("wv", [D, 512], F16, kind="ExternalInput").ap()
    wp_d = nc.dram_tensor("wp", [512, D], F16, kind="ExternalInput").ap()
    cos_d = nc.dram_tensor("cos", [128, T], F16, kind="ExternalInput").ap()
    sin_d = nc.dram_tensor("sin", [128, T], F16, kind="ExternalInput").ap()
    outT_d = nc.dram_tensor("outT", [D, T], F16, kind="ExternalOutput").ap()

    with tile.TileContext(nc) as tc:
        with tc.tile_pool(name="const", bufs=1) as cpool, \
             tc.tile_pool(name="big", bufs=1) as big, \
             tc.tile_pool(name="rope", bufs=2) as rope, \
             tc.tile_pool(name="pbuf", bufs=8) as pbuf, \
             tc.tile_pool(name="rbuf", bufs=2) as rbuf, \
             tc.tile_pool(name="ost", bufs=3) as ost:

            # ---- DMA order: wv + xT feed phase A-v immediately; wqk ct0/
            # ct4 + cos/sin arrive while A-v runs (first aqk units); rest
            # rides behind. ----
            xT_sb = big.tile([128, NKC, T], F16)
            wqk_sb = big.tile([128, NKC, 1024], F16)
            wv_sb = big.tile([128, NKC, 512], F16)
            wp_sb = big.tile([128, NPAIR, 1024], F16)
            cos_sb = cpool.tile([128, T], F16)
            sin_sb = cpool.tile([128, T], F16)
            ones_stripe_done = False

            # upfront compute needs only wv + xT[:, :512] + wqk + cos/sin;
            # later xT chunks feed window-phase A-v filler units.  Inputs
            # are split across the two HWDGE queues (sync carries wv/xT,
            # the idle-at-startup scalar queue carries the weights) so the
            # upfront phase is never DMA-dispatch-bound.
            # single consolidated dispatch per tensor chunk (the strided
            # source AP yields the same 1KB descriptor lines but amortizes
            # the ~0.6us per-dispatch queue cost)
            nc.sync.dma_start(wv_sb[:],
                              wv_d.rearrange("(kc p) c -> p kc c", p=128))
            nc.sync.dma_start(
                xT_sb[:, :, 0:512],
                xT_d[:, 0:512].rearrange("(kc p) c -> p kc c", p=128))
            nc.sync.dma_start(
                xT_sb[:, :, 512:1024],
                xT_d[:, 512:1024].rearrange("(kc p) c -> p kc c", p=128))
            nc.scalar.dma_start(wqk_sb[:],
                                wqk_d.rearrange("(kc p) c -> p kc c", p=128))
            nc.scalar.dma_start(cos_sb[:], cos_d)
            nc.scalar.dma_start(sin_sb[:], sin_d)
            nc.scalar.dma_start(
                xT_sb[:, :, 1024:1536],
                xT_d[:, 1024:1536].rearrange("(kc p) c -> p kc c", p=128))
            nc.scalar.dma_start(
                xT_sb[:, :, 1536:2048],
                xT_d[:, 1536:2048].rearrange("(kc p) c -> p kc c", p=128))
            nc.scalar.dma_start(wp_sb[:],
                                wp_d.rearrange("(cc p) c -> p cc c", p=128))

            v_aug = big.tile([128, NKB, HPC, 65], F16)
            # only the ones-column (col 64 of each head slot) needs init
            nc.vector.memset(v_aug[:, :, :, 64:65], 1.0)

            qkT_sb = big.tile([128, 2 * NPAIR, T], F16)
            y_all = big.tile([128, NPAIR, T], F16)

            # preload the gpsimd library that partition_broadcast needs so
            # the first real broadcast doesn't eat the load latency
            scr_i = cpool.tile([1, 8], F32)
            nc.vector.memset(scr_i[:], 1.0)
            scr_o = cpool.tile([2, 8], F32)
            nc.gpsimd.partition_broadcast(scr_o[:], scr_i[:])

            # ---- upfront PSUM pools (right side): qkps below vps so vps
            # can close first ----
            ph_qk = tc.tile_pool(name="qkps", bufs=1, space="PSUM", side="right")
            qkpsp = ph_qk.__enter__()
            ph_v = tc.tile_pool(name="vps", bufs=2, space="PSUM", side="right")
            vpsp = ph_v.__enter__()

            # ---- HAM warmup: the PE is DMA-blocked for the first ~8us
            # anyway; dummy matmuls on an (uninitialized) scratch tile keep
            # the activity monitor ramping so real work starts at full
            # rate instead of k=4 half-throttle. ----
            wscr = cpool.tile([128, 512], F16)
            nc.vector.memset(wscr[:], 0.5)
            for w in range(24):
                wps = vpsp.tile([128, 512], F32, name=f"warm_{w}", tag="vps")
                nc.tensor.matmul(
                    wps[:], lhsT=wscr[:, 0:128], rhs=wscr[:],
                    start=True, stop=True, skip_group_check=True,
                )

            # ---- A-v unit: v projection for one 128-row t-block, natural
            # [t, d] layout.  Upfront blocks use the vps pool; window-filler
            # blocks borrow a slot of the (shared-tag) qkps ring. ----
            def gen_av(tt, pool, full):
                if full:
                    qt = pool.tile([128, 512], F32,
                                   name=f"avq_{tt}", tag="qkps")
                    vt = qt[:]
                else:
                    vt_t = pool.tile([128, 512], F32,
                                     name=f"vps_{tt}", tag="vps")
                    vt = vt_t[:]
                for kc in range(NKC):
                    nc.tensor.matmul(
                        vt,
                        lhsT=xT_sb[:, kc, tt * 128:(tt + 1) * 128],
                        rhs=wv_sb[:, kc, :],
                        start=(kc == 0), stop=(kc == NKC - 1),
                        skip_group_check=True,
                    )
                    yield 1
                nc.vector.tensor_copy(
                    v_aug[:, tt, :, 0:64],
                    vt.rearrange("p (h d) -> p h d", h=HPC),
                )
                yield 0

            # upfront: the first 6 t-blocks (window (0,0) needs 4; two
            # more so early windows aren't filler-overloaded)
            for tt in range(6):
                for _ in gen_av(tt, vpsp, False):
                    pass

            ph_v.__exit__(None, None, None)  # vps banks -> free

            # ---- A-qk unit: one (ctile, T-half) projection+RoPE, emitted
            # as 16 single matmuls via a generator so it can interleave as
            # PE filler inside attention windows. ----
            def gen_aqk(ct, hf):
                for tcc in range(2):
                    qkps_t = qkpsp.tile([128, 512], F32,
                                        name=f"qkps_{ct}_{hf}_{tcc}",
                                        tag="qkps")
                    for kc in range(NKC):
                        nc.tensor.matmul(
                            qkps_t[:],
                            lhsT=wqk_sb[:, kc, ct * 128:(ct + 1) * 128],
                            rhs=xT_sb[:, kc,
                                      hf * 1024 + tcc * 512:
                                      hf * 1024 + (tcc + 1) * 512],
                            start=(kc == 0), stop=(kc == NKC - 1),
                            skip_group_check=True,
                        )
                        yield 1
                    csl = slice(hf * 1024 + tcc * 512,
                                hf * 1024 + (tcc + 1) * 512)
                    nm = f"{ct}_{hf}_{tcc}"
                    xbf = rope.tile([128, 512], F16, name=f"xbf_{nm}", tag="xbf")
                    nc.vector.tensor_copy(xbf[:], qkps_t[:])
                    ybf = rope.tile([128, 512], F16, name=f"ybf_{nm}", tag="ybf")
                    nc.vector.stream_shuffle(ybf[:], xbf[:], SWAP_MASK)
                    t1 = rope.tile([128, 512], F16, name=f"t1_{nm}", tag="t1")
                    nc.vector.tensor_tensor(t1[:], xbf[:], cos_sb[:, csl],
                                            mybir.AluOpType.mult)
                    t2 = rope.tile([128, 512], F16, name=f"t2_{nm}", tag="t2")
                    nc.vector.tensor_tensor(t2[:], ybf[:], sin_sb[:, csl],
                                            mybir.AluOpType.mult)
                    nc.vector.tensor_add(qkT_sb[:, ct, csl], t1[:], t2[:])
                    yield 0

            opsp = None

            def gen_dot(qc, ot):
                ops_t = opsp.tile([128, 512], F32,
                                  name=f"ops_{qc}_{ot}", tag="ops")
                for pr in range(NPAIR):
                    nc.tensor.matmul(
                        ops_t[:],
                        lhsT=wp_sb[:, pr, ot * 128:(ot + 1) * 128],
                        rhs=y_all[:, pr, qc * 512:(qc + 1) * 512],
                        start=(pr == 0), stop=(pr == NPAIR - 1),
                        skip_group_check=True,
                    )
                    yield 1
                st = ost.tile([128, 512], F16, name=f"st_{qc}_{ot}", tag="st")
                nc.vector.tensor_copy(st[:], ops_t[:])
                nc.sync.dma_start(
                    outT_d[ot * 128:(ot + 1) * 128,
                           qc * 512:(qc + 1) * 512], st[:])
                yield 0

            # filler machinery: an ordered queue of generators; drip pulls
            # a few matmuls at a time, drain-by-name forces completion.
            # supply[0] tracks remaining filler yields, slots[0] remaining
            # kb iterations, so drip spreads filler uniformly over the
            # attention windows (PE load per kb stays level with the exp
            # cadence instead of lurching between féast and famine).
            filler_q = []  # list of [key, generator]
            supply = [0]
            slots = [sum(4 * qc + 4 for qc in range(NQC)) * NPAIR]

            def drip_n():
                if not slots[0]:
                    return 2
                return max(1, min(4, round(supply[0] / slots[0] + 0.8)))

            def drip(n):
                mms = 0
                while mms < n:
                    if not filler_q:
                        return
                    key, g = filler_q[0]
                    try:
                        if next(g):
                            supply[0] -= 1
                            mms += 1
                    except StopIteration:
                        filler_q.pop(0)

            def drain(key):
                # complete units strictly in queue order up to `key` — the
                # single-buffer qkps ring forbids jumping past a
                # partially-consumed unit
                if not any(k == key for k, _ in filler_q):
                    return
                while filler_q:
                    k, g = filler_q.pop(0)
                    for tag in g:
                        supply[0] -= tag
                    if k == key:
                        return

            def drain_all():
                while filler_q:
                    _, g = filler_q.pop(0)
                    for tag in g:
                        supply[0] -= tag

            # pair-0 q/k units run upfront (windows need them immediately)
            for _ in gen_aqk(0, 0):
                pass
            for _ in gen_aqk(4, 0):
                pass
            # remaining A-v blocks + q/k units become window filler,
            # ordered by need
            for tt in range(6, 8):
                filler_q.append([f"av_{tt}", gen_av(tt, qkpsp, True)])
                supply[0] += 8
            filler_q.append(["aqk_0_1", gen_aqk(0, 1)])
            supply[0] += 16
            for tt in range(8, 12):
                filler_q.append([f"av_{tt}", gen_av(tt, qkpsp, True)])
                supply[0] += 8
            filler_q.append(["aqk_4_1", gen_aqk(4, 1)])
            supply[0] += 16
            for tt in range(12, 16):
                filler_q.append([f"av_{tt}", gen_av(tt, qkpsp, True)])
                supply[0] += 8
            for ct, hf in [(1, 0), (5, 0), (1, 1), (5, 1),
                           (2, 0), (6, 0), (2, 1), (6, 1),
                           (3, 0), (7, 0), (3, 1), (7, 1)]:
                filler_q.append([f"aqk_{ct}_{hf}", gen_aqk(ct, hf)])
                supply[0] += 16

            # ---- attention windows (left-side PSUM pools) ----
            ph_s = tc.tile_pool(name="sps", bufs=2, space="PSUM")
            spsp = ph_s.__enter__()
            ph_y = tc.tile_pool(name="ytps", bufs=3, space="PSUM")
            ytpsp = ph_y.__enter__()

            # forced-drain requirements: (pr, qc, kb==None -> window start).
            # A-v filler blocks are forced one score ahead of their PV use;
            # pair-3's q/k units are spread across pair-2 windows so the
            # pair-3 boundary has no serialized drain lump.
            need = {
                (0, 1, 5): ["aqk_0_1"], (0, 2, 6): ["aqk_4_1"],
                (0, 3, 10): ["aqk_1_0"], (0, 3, 12): ["aqk_5_0"],
                (1, 1, 5): ["aqk_1_1"], (1, 2, 6): ["aqk_5_1"],
                (1, 3, 10): ["aqk_2_0"], (1, 3, 12): ["aqk_6_0"],
                (2, 1, 5): ["aqk_2_1"], (2, 2, 6): ["aqk_6_1"],
                (2, 2, 2): ["aqk_3_0"], (2, 2, 8): ["aqk_7_0"],
                (2, 3, 2): ["aqk_3_1"], (2, 3, 8): ["aqk_7_1"],
            }
            for j in range(6, 8):
                need.setdefault((0, 1, j), []).append(f"av_{j}")
            for j in range(8, 12):
                need.setdefault((0, 2, j), []).append(f"av_{j}")
            for j in range(12, 16):
                need.setdefault((0, 3, j), []).append(f"av_{j}")

            def emit_score(pr, qc, kb, sps_t, q_t, k_t):
                off = max(0, (kb - 4 * qc) * 128)
                for h in range(2):
                    nc.tensor.matmul(
                        sps_t[:, h, off:512],
                        lhsT=k_t[h * 64:(h + 1) * 64,
                                 kb * 128:(kb + 1) * 128],
                        rhs=q_t[h * 64:(h + 1) * 64,
                                qc * 512 + off:(qc + 1) * 512],
                        start=True, stop=True,
                        skip_group_check=True,
                    )
                pt = pbuf.tile([128, 2, 512], F16,
                               name=f"pt_{qc}_{pr}_{kb}", tag="pt")
                nc.scalar.activation(
                    pt[:, :, off:512], sps_t[:, :, off:512],
                    mybir.ActivationFunctionType.Exp, scale=0.125)
                if kb >= 4 * qc:  # diagonal block: triangular mask
                    for h in range(2):
                        nc.gpsimd.affine_select(
                            out=pt[:, h, off:off + 128],
                            in_=pt[:, h, off:off + 128],
                            compare_op=mybir.AluOpType.is_ge,
                            fill=0.0, base=0,
                            pattern=[[1, 128]],
                            channel_multiplier=-1)
                return pt, off

            dot_ready = []  # DOT units whose y inputs are complete
            pending = [None]  # previous window's finisher closure

            for pr in range(NPAIR):
                q_t = qkT_sb[:, pr, :]
                k_t = qkT_sb[:, NPAIR + pr, :]
                if pr == 3:
                    # finish (2,3) before its yt pool closes
                    if pending[0] is not None:
                        pending[0]()
                        pending[0] = None
                    # all projection filler must finish; rebalance PSUM:
                    # shrink the yt ring back to 2 and hand its bank plus
                    # the qkps bank to the out-projection pool
                    drain_all()
                    ph_qk.__exit__(None, None, None)
                    ph_y.__exit__(None, None, None)
                    ph_y2 = tc.tile_pool(name="ytps2", bufs=2, space="PSUM")
                    ytpsp = ph_y2.__enter__()
                    ph_o = tc.tile_pool(name="ops", bufs=2, space="PSUM")
                    opsp = ph_o.__enter__()
                for qc in range(NQC):
                    for k in need.get((pr, qc, None), []):
                        drain(k)
                    nkb = 4 * qc + 4
                    sc0 = {}
                    sps0 = spsp.tile([128, 2, 512], F32,
                                     name=f"sps_{qc}_{pr}_0", tag="sps")
                    sc0[0] = emit_score(pr, qc, 0, sps0, q_t, k_t)
                    if pending[0] is not None:
                        pending[0]()
                        pending[0] = None
                    yt0 = ytpsp.tile([65, 512], F32,
                                     name=f"yt0_{qc}_{pr}", tag="yt")
                    yt1 = ytpsp.tile([65, 512], F32,
                                     name=f"yt1_{qc}_{pr}", tag="yt")
                    yts = (yt0, yt1)
                    # software pipeline: PV lags the scores by one block so
                    # every PE instruction's inputs (exp, affine, yt slot)
                    # are already complete when it issues — no micro-waits,
                    # the PE power-ramp stays hot.  Filler matmuls drip in
                    # between; extra at window start to cover the previous
                    # window's normalize chain releasing the yt slots.
                    sc = sc0

                    def emit_pv(kb, sc=sc, yts=yts, pr=pr, nkb=nkb):
                        pt, off = sc.pop(kb)
                        for h in range(2):
                            nc.tensor.matmul(
                                yts[h][:, off:512],
                                lhsT=v_aug[:, kb, 2 * pr + h, :],
                                rhs=pt[:, h, off:512],
                                start=(kb == 0), stop=(kb == nkb - 1),
                                skip_group_check=True,
                            )

                    for kb in range(nkb):
                        for k in need.get((pr, qc, kb + 1), []):
                            drain(k)
                        if kb + 1 < nkb:
                            spsn = spsp.tile([128, 2, 512], F32,
                                             name=f"sps_{qc}_{pr}_{kb+1}",
                                             tag="sps")
                            sc[kb + 1] = emit_score(pr, qc, kb + 1, spsn,
                                                    q_t, k_t)
                        drip(drip_n())
                        slots[0] -= 1
                        if kb >= 1:
                            emit_pv(kb - 1)

                    def finish(pr=pr, qc=qc, nkb=nkb, yts=yts,
                               emit_pv=emit_pv):
                        emit_pv(nkb - 1)
                        # normalize: 1/l via DVE reciprocal, gpsimd
                        # partition-broadcast, one fused multiply into y_all
                        for h in range(2):
                            lsb = rbuf.tile([1, 512], F32,
                                            name=f"lsb_{qc}_{pr}_{h}",
                                            tag="lsb")
                            nc.vector.tensor_copy(lsb[:], yts[h][64:65, :])
                            rr = rbuf.tile([1, 512], F32,
                                           name=f"rr_{qc}_{pr}_{h}", tag="rr")
                            nc.vector.reciprocal_approx_fast(rr[:], lsb[:])
                            rbc = rbuf.tile([64, 512], F32,
                                            name=f"rbc_{qc}_{pr}_{h}",
                                            tag="rbc")
                            nc.gpsimd.partition_broadcast(rbc[:], rr[:])
                            nc.vector.tensor_tensor(
                                y_all[h * 64:(h + 1) * 64, pr,
                                      qc * 512:(qc + 1) * 512],
                                yts[h][0:64, :], rbc[:],
                                mybir.AluOpType.mult)
                        if pr == 3:
                            g = [[f"dot_{qc}_{ot}", gen_dot(qc, ot)]
                                 for ot in range(8)]
                            if qc < NQC - 1:
                                filler_q.extend(g)
                                supply[0] += 32
                            else:
                                dot_ready.extend(g)

                    pending[0] = finish

            pending[0]()
            pending[0] = None
            drain_all()
            # the final out-projection units run after the last window; the
            # yt banks are free now, so give them a deep ops ring — the
            # tail drains at PE speed instead of serializing on st-copies
            ph_o.__exit__(None, None, None)
            ph_y2.__exit__(None, None, None)
            ph_o2 = tc.tile_pool(name="ops2", bufs=4, space="PSUM")
            opsp = ph_o2.__enter__()
            for _, g in dot_ready:
                for _ in g:
                    pass

            ph_o2.__exit__(None, None, None)
            ph_s.__exit__(None, None, None)

    nc.compile()
    return nc


def _host_inputs(x, W_attn, W_proj):
    """Build the per-core input maps (host-side shard + layout prep)."""
    j = np.arange(16)
    perm = np.concatenate([2 * j, 2 * j + 1, 32 + 2 * j, 33 + 2 * j])

    # RoPE tables in the permuted-transposed layout, fp32 math then fp16.
    inv_freq = 1.0 / (ROPE_BASE ** (np.arange(0, D_HEAD, 2, dtype=np.float64)
                                    / D_HEAD))  # [32]
    t = np.arange(T, dtype=np.float64)
    freqs = np.outer(inv_freq, t)  # [32, T]
    jmap = np.concatenate([j, j, 16 + j, 16 + j])  # per-head 64 rows
    jmap = np.concatenate([jmap, jmap])  # 128 rows (2 heads)
    sign = np.tile(np.concatenate([-np.ones(16), np.ones(16)]), 4)  # [128]
    cos_tab = np.cos(freqs[jmap]).astype(np.float16)
    sin_tab = (sign[:, None] * np.sin(freqs[jmap])).astype(np.float16)

    in_maps = []
    for c in range(N_CORES):
        b, half = divmod(c, 2)
        heads = [8 * half + i for i in range(HPC)]
        # wqk: 4 q-pair ctiles then 4 k-pair ctiles, per-head perm'd cols
        cols = []
        for base in (0, D):  # q block, k block of W_attn
            for hp in range(NPAIR):
                for g in (heads[2 * hp], heads[2 * hp + 1]):
                    cols.append(base + g * D_HEAD + perm)
        wqk = W_attn[:, np.concatenate(cols)].astype(np.float16)
        wv = W_attn[:, 2 * D + 512 * half: 2 * D + 512 * (half + 1)] \
            .astype(np.float16)
        wp = W_proj[512 * half: 512 * (half + 1), :].astype(np.float16)
        xT = np.ascontiguousarray(x[b].T).astype(np.float16)
        in_maps.append({
            "xT": xT, "wqk": wqk, "wv": wv, "wp": wp,
            "cos": cos_tab, "sin": sin_tab,
        })
    return in_maps


_NC_CACHE = None


def kernel(x, W_attn, W_proj, _trace=False):
    global _NC_CACHE
    x = np.asarray(x, dtype=np.float32)
    W_attn = np.asarray(W_attn, dtype=np.float32)
    W_proj = np.asarray(W_proj, dtype=np.float32)

    if _NC_CACHE is None:
        _NC_CACHE = _build_program()
    nc = _NC_CACHE

    in_maps = _host_inputs(x, W_attn, W_proj)
    res = run_bass_kernel_spmd(nc, in_maps, core_ids=list(range(N_CORES)),
                               trace=_trace)

    y = np.empty((B, T, D), dtype=np.float32)
    for b in range(B):
        y[b] = (res.results[2 * b]["outT"].astype(np.float32)
                + res.results[2 * b + 1]["outT"].astype(np.float32)).T
    if _trace:
        return y, res
    return y


# revision 32
# speedup vs baseline: 1.1752x; 1.1752x over previous
"""Causal self-attention with RoPE, fused Trainium2 Bass kernel, 8 NeuronCores.

Problem: x[4,2048,1024] @ W_attn[1024,3072] -> qkv; RoPE(q,k); causal
softmax attention (16 heads, d=64); y @ W_proj[1024,1024].

Sharding (data + head parallel): core c handles batch b=c//2 and heads
8*(c%2)..8*(c%2)+7.  W_attn is column-sharded by head, W_proj row-sharded;
each core emits a partial output projection and the host sums the two
partials per batch (the 2-way "all-reduce").

Kernel layout choices (per core):
 - Everything transposed: xT [D,T] in SBUF, q/k produced as qT/kT [d,T],
   attention computed as scoresT [k,q] so softmax-sum and PV contraction
   both run along the partition axis via matmuls (no transposes needed).
 - RoPE: head-dim channels are pre-permuted (via W_attn column permutation)
   into [e0..e15, o0..o15, e16..e31, o16..o31] per head so the rotate-half
   pairing is a 16<->16 swap inside each 32-partition quadrant, done with a
   single DVE stream_shuffle.  cos/sin tables (sign-folded) come from host.
 - No max-subtraction in softmax: scores/8 are ~N(0,0.4), exp is safe.
   l (row sum) comes free by appending a ones column to V (M=65 PV matmul).
 - fp16 operands everywhere on the PE (full-rate); fp32 accumulation.
 - Scores matmuls for a head pair run concurrently via row-tiled PE
   (stationaries at base partitions 0/64, separate PSUM banks).

Scheduling (v2, PE-bound regime):
 - All of phase A-v (x@Wv) runs upfront: the ACT engine is idle then
   anyway, and it frees its PSUM bank for the window phase.
 - The ACT queue carries ONLY the softmax exps; everything else
   (copies, casts) lives on DVE/GPSIMD so window boundaries never stall
   the exp stream.
 - Attention windows are software-pipelined: score(kb+1) is emitted
   before PV(kb), with filler matmuls (remaining q/k projection units,
   out-projection units) dripped between score and PV so the PE chews
   dense work while ACT runs exp.
 - Normalization: 1/l via DVE reciprocal straight from PSUM row 64,
   broadcast across partitions with a GPSIMD partition_broadcast, then a
   single fused DVE multiply (PSUM y x SBUF r -> SBUF f16).  No PE
   broadcast matmul, nothing on ACT.
 - Output partials are f16 (halves the store DMA); host sums in f32.
"""

import sys

sys.path.insert(0, "/opt/trn_rl_repo")

import numpy as np

import concourse.bass as bass  # noqa: F401  (import registers engine classes)
import concourse.mybir as mybir
import concourse.tile as tile
from concourse import bacc
from concourse.bass_utils import run_bass_kernel_spmd

F16 = mybir.dt.float16
F32 = mybir.dt.float32

B, T, D = 4, 2048, 1024
N_HEAD, D_HEAD = 16, 64
ROPE_BASE = 10000.0
N_CORES = 8
HPC = N_HEAD // 2  # heads per core (8)
NPAIR = HPC // 2  # head pairs per core (4)
NKC = D // 128  # k-chunks (8)
NQC = T // 512  # q chunks of 512 (4)
NKB = T // 128  # k blocks of 128 (16)

SWAP_MASK = list(range(16, 32)) + list(range(0, 16))


def _build_program():
    nc = bacc.Bacc("TRN2", target_bir_lowering=False, debug=False,
                   num_devices=N_CORES)

    xT_d = nc.dram_tensor("xT", [D, T], F16, kind="ExternalInput").ap()
    wqk_d = nc.dram_tensor("wqk", [D, 1024], F16, kind="ExternalInput").ap()
    wv_d = nc.dram_tensor("wv", [D, 512], F16, kind="ExternalInput").ap()
    wp_d = nc.dram_tensor("wp", [512, D], F16, kind="ExternalInput").ap()
    cos_d = nc.dram_tensor("cos", [128, T], F16, kind="ExternalInput").ap()
    sin_d = nc.dram_tensor("sin", [128, T], F16, kind="ExternalInput").ap()
    outT_d = nc.dram_tensor("outT", [D, T], F16, kind="ExternalOutput").ap()

    with tile.TileContext(nc) as tc:
        with tc.tile_pool(name="const", bufs=1) as cpool, \
             tc.tile_pool(name="big", bufs=1) as big, \
             tc.tile_pool(name="rope", bufs=2) as rope, \
             tc.tile_pool(name="pbuf", bufs=8) as pbuf, \
             tc.tile_pool(name="rbuf", bufs=2) as rbuf, \
             tc.tile_pool(name="ost", bufs=3) as ost:

            # ---- DMA order: wv + xT feed phase A-v immediately; wqk ct0/
            # ct4 + cos/sin arrive while A-v runs (first aqk units); rest
            # rides behind. ----
            xT_sb = big.tile([128, NKC, T], F16)
            wqk_sb = big.tile([128, NKC, 1024], F16)
            wv_sb = big.tile([128, NKC, 512], F16)
            wp_sb = big.tile([128, NPAIR, 1024], F16)
            cos_sb = cpool.tile([128, T], F16)
            sin_sb = cpool.tile([128, T], F16)
            ones_stripe_done = False

            # upfront compute needs only wv + xT[:, :512] + wqk + cos/sin;
            # later xT chunks feed window-phase A-v filler units.  Inputs
            # are split across the two HWDGE queues (sync carries wv/xT,
            # the idle-at-startup scalar queue carries the weights) so the
            # upfront phase is never DMA-dispatch-bound.
            # single consolidated dispatch per tensor chunk (the strided
            # source AP yields the same 1KB descriptor lines but amortizes
            # the ~0.6us per-dispatch queue cost)
            nc.sync.dma_start(wv_sb[:],
                              wv_d.rearrange("(kc p) c -> p kc c", p=128))
            nc.sync.dma_start(
                xT_sb[:, :, 0:512],
                xT_d[:, 0:512].rearrange("(kc p) c -> p kc c", p=128))
            nc.sync.dma_start(
                xT_sb[:, :, 512:1024],
                xT_d[:, 512:1024].rearrange("(kc p) c -> p kc c", p=128))
            nc.scalar.dma_start(wqk_sb[:],
                                wqk_d.rearrange("(kc p) c -> p kc c", p=128))
            nc.scalar.dma_start(cos_sb[:], cos_d)
            nc.scalar.dma_start(sin_sb[:], sin_d)
            nc.scalar.dma_start(
                xT_sb[:, :, 1024:1536],
                xT_d[:, 1024:1536].rearrange("(kc p) c -> p kc c", p=128))
            nc.scalar.dma_start(
                xT_sb[:, :, 1536:2048],
                xT_d[:, 1536:2048].rearrange("(kc p) c -> p kc c", p=128))
            nc.scalar.dma_start(wp_sb[:],
                                wp_d.rearrange("(cc p) c -> p cc c", p=128))

            v_aug = big.tile([128, NKB, HPC, 65], F16)
            # only the ones-column (col 64 of each head slot) needs init
            nc.vector.memset(v_aug[:, :, :, 64:65], 1.0)

            qkT_sb = big.tile([128, 2 * NPAIR, T], F16)
            y_all = big.tile([128, NPAIR, T], F16)

            # preload the gpsimd library that partition_broadcast needs so
            # the first real broadcast doesn't eat the load latency
            scr_i = cpool.tile([1, 8], F32)
            nc.vector.memset(scr_i[:], 1.0)
            scr_o = cpool.tile([2, 8], F32)
            nc.gpsimd.partition_broadcast(scr_o[:], scr_i[:])

            # ---- upfront PSUM pools (right side): qkps below vps so vps
            # can close first ----
            ph_qk = tc.tile_pool(name="qkps", bufs=1, space="PSUM", side="right")
            qkpsp = ph_qk.__enter__()
            ph_v = tc.tile_pool(name="vps", bufs=2, space="PSUM", side="right")
            vpsp = ph_v.__enter__()

            # ---- HAM warmup: the PE is DMA-blocked for the first ~8us
            # anyway; dummy matmuls on an (uninitialized) scratch tile keep
            # the activity monitor ramping so real work starts at full
            # rate instead of k=4 half-throttle. ----
            wscr = cpool.tile([128, 512], F16)
            nc.vector.memset(wscr[:], 0.5)
            for w in range(24):
                wps = vpsp.tile([128, 512], F32, name=f"warm_{w}", tag="vps")
                nc.tensor.matmul(
                    wps[:], lhsT=wscr[:, 0:128], rhs=wscr[:],
                    start=True, stop=True, skip_group_check=True,
                )

            # ---- A-v unit: v projection for one 128-row t-block, natural
            # [t, d] layout.  Upfront blocks use the vps pool; window-filler
            # blocks borrow a slot of the (shared-tag) qkps ring. ----
            def gen_av(tt, pool, full):
                if full:
                    qt = pool.tile([128, 512], F32,
                                   name=f"avq_{tt}", tag="qkps")
                    vt = qt[:]
                else:
                    vt_t = pool.tile([128, 512], F32,
                                     name=f"vps_{tt}", tag="vps")
                    vt = vt_t[:]
                for kc in range(NKC):
                    nc.tensor.matmul(
                        vt,
                        lhsT=xT_sb[:, kc, tt * 128:(tt + 1) * 128],
                        rhs=wv_sb[:, kc, :],
                        start=(kc == 0), stop=(kc == NKC - 1),
                        skip_group_check=True,
                    )
                    yield 1
                nc.vector.tensor_copy(
                    v_aug[:, tt, :, 0:64],
                    vt.rearrange("p (h d) -> p h d", h=HPC),
                )
                yield 0

            # upfront: the first 6 t-blocks (window (0,0) needs 4; two
            # more so early windows aren't filler-overloaded)
            for tt in range(6):
                for _ in gen_av(tt, vpsp, False):
                    pass

            ph_v.__exit__(None, None, None)  # vps banks -> free

            # ---- A-qk unit: one (ctile, T-half) projection+RoPE, emitted
            # as 16 single matmuls via a generator so it can interleave as
            # PE filler inside attention windows. ----
            def gen_aqk(ct, hf):
                for tcc in range(2):
                    qkps_t = qkpsp.tile([128, 512], F32,
                                        name=f"qkps_{ct}_{hf}_{tcc}",
                                        tag="qkps")
                    for kc in range(NKC):
                        nc.tensor.matmul(
                            qkps_t[:],
                            lhsT=wqk_sb[:, kc, ct * 128:(ct + 1) * 128],
                            rhs=xT_sb[:, kc,
                                      hf * 1024 + tcc * 512:
                                      hf * 1024 + (tcc + 1) * 512],
                            start=(kc == 0), stop=(kc == NKC - 1),
                            skip_group_check=True,
                        )
                        yield 1
                    csl = slice(hf * 1024 + tcc * 512,
                                hf * 1024 + (tcc + 1) * 512)
                    nm = f"{ct}_{hf}_{tcc}"
                    xbf = rope.tile([128, 512], F16, name=f"xbf_{nm}", tag="xbf")
                    nc.vector.tensor_copy(xbf[:], qkps_t[:])
                    ybf = rope.tile([128, 512], F16, name=f"ybf_{nm}", tag="ybf")
                    nc.vector.stream_shuffle(ybf[:], xbf[:], SWAP_MASK)
                    t1 = rope.tile([128, 512], F16, name=f"t1_{nm}", tag="t1")
                    nc.vector.tensor_tensor(t1[:], xbf[:], cos_sb[:, csl],
                                            mybir.AluOpType.mult)
                    t2 = rope.tile([128, 512], F16, name=f"t2_{nm}", tag="t2")
                    nc.vector.tensor_tensor(t2[:], ybf[:], sin_sb[:, csl],
                                            mybir.AluOpType.mult)
                    nc.vector.tensor_add(qkT_sb[:, ct, csl], t1[:], t2[:])
                    yield 0

            opsp = None

            def gen_dot(qc, ot):
                ops_t = opsp.tile([128, 512], F32,
                                  name=f"ops_{qc}_{ot}", tag="ops")
                for pr in range(NPAIR):
                    nc.tensor.matmul(
                        ops_t[:],
                        lhsT=wp_sb[:, pr, ot * 128:(ot + 1) * 128],
                        rhs=y_all[:, pr, qc * 512:(qc + 1) * 512],
                        start=(pr == 0), stop=(pr == NPAIR - 1),
                        skip_group_check=True,
                    )
                    yield 1
                st = ost.tile([128, 512], F16, name=f"st_{qc}_{ot}", tag="st")
                nc.vector.tensor_copy(st[:], ops_t[:])
                nc.sync.dma_start(
                    outT_d[ot * 128:(ot + 1) * 128,
                           qc * 512:(qc + 1) * 512], st[:])
                yield 0

            # filler machinery: an ordered queue of generators; drip pulls
            # a few matmuls at a time, drain-by-name forces completion.
            # supply[0] tracks remaining filler yields, slots[0] remaining
            # kb iterations, so drip spreads filler uniformly over the
            # attention windows (PE load per kb stays level with the exp
            # cadence instead of lurching between féast and famine).
            filler_q = []  # list of [key, generator]
            supply = [0]
            slots = [sum(4 * qc + 4 for qc in range(NQC)) * NPAIR]

            def drip_n():
                if not slots[0]:
                    return 2
                return max(1, min(4, round(supply[0] / slots[0] + 0.3)))

            def drip(n):
                mms = 0
                while mms < n:
                    if not filler_q:
                        return
                    key, g = filler_q[0]
                    try:
                        if next(g):
                            supply[0] -= 1
                            mms += 1
                    except StopIteration:
                        filler_q.pop(0)

            def drain(key):
                # complete units strictly in queue order up to `key` — the
                # single-buffer qkps ring forbids jumping past a
                # partially-consumed unit
                if not any(k == key for k, _ in filler_q):
                    return
                while filler_q:
                    k, g = filler_q.pop(0)
                    for tag in g:
                        supply[0] -= tag
                    if k == key:
                        return

            def drain_all():
                while filler_q:
                    _, g = filler_q.pop(0)
                    for tag in g:
                        supply[0] -= tag

            # pair-0 q/k units run upfront (windows need them immediately)
            for _ in gen_aqk(0, 0):
                pass
            for _ in gen_aqk(4, 0):
                pass
            # remaining A-v blocks + q/k units become window filler,
            # ordered by need
            for tt in range(6, 8):
                filler_q.append([f"av_{tt}", gen_av(tt, qkpsp, True)])
                supply[0] += 8
            filler_q.append(["aqk_0_1", gen_aqk(0, 1)])
            supply[0] += 16
            for tt in range(8, 12):
                filler_q.append([f"av_{tt}", gen_av(tt, qkpsp, True)])
                supply[0] += 8
            filler_q.append(["aqk_4_1", gen_aqk(4, 1)])
            supply[0] += 16
            for tt in range(12, 16):
                filler_q.append([f"av_{tt}", gen_av(tt, qkpsp, True)])
                supply[0] += 8
            for ct, hf in [(1, 0), (5, 0), (1, 1), (5, 1),
                           (2, 0), (6, 0), (2, 1), (6, 1),
                           (3, 0), (7, 0), (3, 1), (7, 1)]:
                filler_q.append([f"aqk_{ct}_{hf}", gen_aqk(ct, hf)])
                supply[0] += 16

            # ---- attention windows (left-side PSUM pools) ----
            ph_s = tc.tile_pool(name="sps", bufs=2, space="PSUM")
            spsp = ph_s.__enter__()
            ph_y = tc.tile_pool(name="ytps", bufs=3, space="PSUM")
            ytpsp = ph_y.__enter__()

            # forced-drain requirements: (pr, qc, kb==None -> window start).
            # A-v filler blocks are forced one score ahead of their PV use;
            # pair-3's q/k units are spread across pair-2 windows so the
            # pair-3 boundary has no serialized drain lump.
            need = {
                (0, 1, 5): ["aqk_0_1"], (0, 2, 6): ["aqk_4_1"],
                (0, 3, 10): ["aqk_1_0"], (0, 3, 12): ["aqk_5_0"],
                (1, 1, 5): ["aqk_1_1"], (1, 2, 6): ["aqk_5_1"],
                (1, 3, 10): ["aqk_2_0"], (1, 3, 12): ["aqk_6_0"],
                (2, 1, 5): ["aqk_2_1"], (2, 2, 6): ["aqk_6_1"],
                (2, 2, 2): ["aqk_3_0"], (2, 2, 8): ["aqk_7_0"],
                (2, 3, 2): ["aqk_3_1"], (2, 3, 8): ["aqk_7_1"],
            }
            for j in range(6, 8):
                need.setdefault((0, 1, j), []).append(f"av_{j}")
            for j in range(8, 12):
                need.setdefault((0, 2, j), []).append(f"av_{j}")
            for j in range(12, 16):
                need.setdefault((0, 3, j), []).append(f"av_{j}")

            def emit_score(pr, qc, kb, sps_t, q_t, k_t):
                off = max(0, (kb - 4 * qc) * 128)
                for h in range(2):
                    nc.tensor.matmul(
                        sps_t[:, h, off:512],
                        lhsT=k_t[h * 64:(h + 1) * 64,
                                 kb * 128:(kb + 1) * 128],
                        rhs=q_t[h * 64:(h + 1) * 64,
                                qc * 512 + off:(qc + 1) * 512],
                        start=True, stop=True,
                        skip_group_check=True,
                    )
                pt = pbuf.tile([128, 2, 512], F16,
                               name=f"pt_{qc}_{pr}_{kb}", tag="pt")
                nc.scalar.activation(
                    pt[:, :, off:512], sps_t[:, :, off:512],
                    mybir.ActivationFunctionType.Exp, scale=0.125)
                if kb >= 4 * qc:  # diagonal block: triangular mask
                    for h in range(2):
                        nc.gpsimd.affine_select(
                            out=pt[:, h, off:off + 128],
                            in_=pt[:, h, off:off + 128],
                            compare_op=mybir.AluOpType.is_ge,
                            fill=0.0, base=0,
                            pattern=[[1, 128]],
                            channel_multiplier=-1)
                return pt, off

            dot_ready = []  # DOT units whose y inputs are complete
            pending = [None]  # previous window's finisher closure

            for pr in range(NPAIR):
                q_t = qkT_sb[:, pr, :]
                k_t = qkT_sb[:, NPAIR + pr, :]
                if pr == 3:
                    # finish (2,3) before its yt pool closes
                    if pending[0] is not None:
                        pending[0]()
                        pending[0] = None
                    # all projection filler must finish; rebalance PSUM:
                    # shrink the yt ring back to 2 and hand its bank plus
                    # the qkps bank to the out-projection pool
                    drain_all()
                    ph_qk.__exit__(None, None, None)
                    ph_y.__exit__(None, None, None)
                    ph_y2 = tc.tile_pool(name="ytps2", bufs=2, space="PSUM")
                    ytpsp = ph_y2.__enter__()
                    ph_o = tc.tile_pool(name="ops", bufs=2, space="PSUM")
                    opsp = ph_o.__enter__()
                for qc in range(NQC):
                    for k in need.get((pr, qc, None), []):
                        drain(k)
                    nkb = 4 * qc + 4
                    sc0 = {}
                    sps0 = spsp.tile([128, 2, 512], F32,
                                     name=f"sps_{qc}_{pr}_0", tag="sps")
                    sc0[0] = emit_score(pr, qc, 0, sps0, q_t, k_t)
                    if pending[0] is not None:
                        pending[0]()
                        pending[0] = None
                    yt0 = ytpsp.tile([65, 512], F32,
                                     name=f"yt0_{qc}_{pr}", tag="yt")
                    yt1 = ytpsp.tile([65, 512], F32,
                                     name=f"yt1_{qc}_{pr}", tag="yt")
                    yts = (yt0, yt1)
                    # software pipeline: PV lags the scores by one block so
                    # every PE instruction's inputs (exp, affine, yt slot)
                    # are already complete when it issues — no micro-waits,
                    # the PE power-ramp stays hot.  Filler matmuls drip in
                    # between; extra at window start to cover the previous
                    # window's normalize chain releasing the yt slots.
                    sc = sc0

                    def emit_pv(kb, sc=sc, yts=yts, pr=pr, nkb=nkb):
                        pt, off = sc.pop(kb)
                        for h in range(2):
                            nc.tensor.matmul(
                                yts[h][:, off:512],
                                lhsT=v_aug[:, kb, 2 * pr + h, :],
                                rhs=pt[:, h, off:512],
                                start=(kb == 0), stop=(kb == nkb - 1),
                                skip_group_check=True,
                            )

                    for kb in range(nkb):
                        for k in need.get((pr, qc, kb + 1), []):
                            drain(k)
                        if kb + 1 < nkb:
                            spsn = spsp.tile([128, 2, 512], F32,
                                             name=f"sps_{qc}_{pr}_{kb+1}",
                                             tag="sps")
                            sc[kb + 1] = emit_score(pr, qc, kb + 1, spsn,
                                                    q_t, k_t)
                        drip(drip_n())
                        slots[0] -= 1
                        if kb >= 1:
                            emit_pv(kb - 1)

                    def finish(pr=pr, qc=qc, nkb=nkb, yts=yts,
                               emit_pv=emit_pv):
                        emit_pv(nkb - 1)
                        # normalize: 1/l via DVE reciprocal, gpsimd
                        # partition-broadcast, one fused multiply into y_all
                        for h in range(2):
                            lsb = rbuf.tile([1, 512], F32,
                                            name=f"lsb_{qc}_{pr}_{h}",
                                            tag="lsb")
                            nc.vector.tensor_copy(lsb[:], yts[h][64:65, :])
                            rr = rbuf.tile([1, 512], F32,
                                           name=f"rr_{qc}_{pr}_{h}", tag="rr")
                            nc.vector.reciprocal_approx_fast(rr[:], lsb[:])
                            rbc = rbuf.tile([64, 512], F32,
                                            name=f"rbc_{qc}_{pr}_{h}",
                                            tag="rbc")
                            nc.gpsimd.partition_broadcast(rbc[:], rr[:])
                            nc.vector.tensor_tensor(
                                y_all[h * 64:(h + 1) * 64, pr,
                                      qc * 512:(qc + 1) * 512],
                                yts[h][0:64, :], rbc[:],
                                mybir.AluOpType.mult)
                        if pr == 3:
                            g = [[f"dot_{qc}_{ot}", gen_dot(qc, ot)]
                                 for ot in range(8)]
                            if qc < NQC - 1:
                                filler_q.extend(g)
                                supply[0] += 32
                            else:
                                dot_ready.extend(g)

                    pending[0] = finish

            pending[0]()
            pending[0] = None
            drain_all()
            # the final out-projection units run after the last window; the
            # yt banks are free now, so give them a deep ops ring — the
            # tail drains at PE speed instead of serializing on st-copies
            ph_o.__exit__(None, None, None)
            ph_y2.__exit__(None, None, None)
            ph_o2 = tc.tile_pool(name="ops2", bufs=4, space="PSUM")
            opsp = ph_o2.__enter__()
            for _, g in dot_ready:
                for _ in g:
                    pass

            ph_o2.__exit__(None, None, None)
            ph_s.__exit__(None, None, None)

    nc.compile()
    return nc


def _host_inputs(x, W_attn, W_proj):
    """Build the per-core input maps (host-side shard + layout prep)."""
    j = np.arange(16)
    perm = np.concatenate([2 * j, 2 * j + 1, 32 + 2 * j, 33 + 2 * j])

    # RoPE tables in the permuted-transposed layout, fp32 math then fp16.
    inv_freq = 1.0 / (ROPE_BASE ** (np.arange(0, D_HEAD, 2, dtype=np.float64)
                                    / D_HEAD))  # [32]
    t = np.arange(T, dtype=np.float64)
    freqs = np.outer(inv_freq, t)  # [32, T]
    jmap = np.concatenate([j, j, 16 + j, 16 + j])  # per-head 64 rows
    jmap = np.concatenate([jmap, jmap])  # 128 rows (2 heads)
    sign = np.tile(np.concatenate([-np.ones(16), np.ones(16)]), 4)  # [128]
    cos_tab = np.cos(freqs[jmap]).astype(np.float16)
    sin_tab = (sign[:, None] * np.sin(freqs[jmap])).astype(np.float16)

    in_maps = []
    for c in range(N_CORES):
        b, half = divmod(c, 2)
        heads = [8 * half + i for i in range(HPC)]
        # wqk: 4 q-pair ctiles then 4 k-pair ctiles, per-head perm'd cols
        cols = []
        for base in (0, D):  # q block, k block of W_attn
            for hp in range(NPAIR):
                for g in (heads[2 * hp], heads[2 * hp + 1]):
                    cols.append(base + g * D_HEAD + perm)
        wqk = W_attn[:, np.concatenate(cols)].astype(np.float16)
        wv = W_attn[:, 2 * D + 512 * half: 2 * D + 512 * (half + 1)] \
            .astype(np.float16)
        wp = W_proj[512 * half: 512 * (half + 1), :].astype(np.float16)
        xT = np.ascontiguousarray(x[b].T).astype(np.float16)
        in_maps.append({
            "xT": xT, "wqk": wqk, "wv": wv, "wp": wp,
            "cos": cos_tab, "sin": sin_tab,
        })
    return in_maps


_NC_CACHE = None


def kernel(x, W_attn, W_proj, _trace=False):
    global _NC_CACHE
    x = np.asarray(x, dtype=np.float32)
    W_attn = np.asarray(W_attn, dtype=np.float32)
    W_proj = np.asarray(W_proj, dtype=np.float32)

    if _NC_CACHE is None:
        _NC_CACHE = _build_program()
    nc = _NC_CACHE

    in_maps = _host_inputs(x, W_attn, W_proj)
    res = run_bass_kernel_spmd(nc, in_maps, core_ids=list(range(N_CORES)),
                               trace=_trace)

    y = np.empty((B, T, D), dtype=np.float32)
    for b in range(B):
        y[b] = (res.results[2 * b]["outT"].astype(np.float32)
                + res.results[2 * b + 1]["outT"].astype(np.float32)).T
    if _trace:
        return y, res
    return y
